# revision 2
# baseline (speedup 1.0000x reference)
"""MoE grouped linear (DMoELinear) on 8 Trainium2 NeuronCores.

Expert-parallel sharding: tokens are sorted by expert id, so expert e's
tokens form one contiguous slice. Core e receives expert e's tokens
(padded to a uniform capacity C = max group size, so all cores run one
SPMD NEFF), expert e's weight and bias, and computes
    yT_e = (x_e @ W_e.T).T.bf16 + b_e.bf16
with the weight block as the stationary matmul operand and tokens as
the moving free dim.

v2 schedule (trace-driven): the run is paced by DMA in the first third
and by the PE after, with a fixed ~8us NEFF epilogue. So: few, large
DMAs (each dma_start costs ~0.6us of issuing-engine time regardless of
size), deadline-ordered across the two HWDGE rings; weights flat
[128, DB*KT*128] so multi-block transfers are single 2D slices; w0/w1
split into k-slices only where the PE needs them early; remaining
weights as 2-block 512KB packs; x as 8 full-width k-tiles alternating
rings. db0/db1 interleave k-steps during the x trickle. Bias add fuses
into PSUM eviction (ACT/DVE alternating).
"""

import numpy as np
import ml_dtypes

N_TOK, D_IN, D_OUT, N_EXP = 8192, 1024, 2048, 8
N_CORES = 8
P = 128
NFREE = 512  # max matmul moving free dim (one PSUM bank of f32)

BF16 = ml_dtypes.bfloat16

_nc_cache: dict[int, object] = {}


def _chunks(C):
    out = []
    off = 0
    while off < C:
        cw = min(NFREE, C - off)
        out.append((off, cw))
        off += cw
    return out


def _build_bass(C: int):
    """Emit the per-core Bass/Tile kernel for token capacity C."""
    import concourse.bass as bass  # noqa: F401  (registers engines)
    import concourse.mybir as mybir
    import concourse.tile as tile
    from concourse import bacc

    dt = mybir.dt
    KT = D_IN // P      # 8 contraction tiles
    DB = D_OUT // P     # 16 output-row blocks
    KW = KT * P         # columns per db block in the flat weight (1024)
    chunks = _chunks(C)
    chunk_of_db = {db: chunks for db in range(DB)}

    nc = bacc.Bacc("TRN2", target_bir_lowering=False)

    # x k-tile ki lives in rows [ki*128, (ki+1)*128).
    xT_d = nc.dram_tensor("xT", [D_IN, C], dt.bfloat16, kind="ExternalInput")
    # flat weights: row p, col db*1024 + kt*128 + d  (lhsT slices are
    # contiguous 128-col blocks; multi-db packs are contiguous too).
    wf_d = nc.dram_tensor("wf", [P, DB * KW], dt.bfloat16, kind="ExternalInput")
    bias_d = nc.dram_tensor("biasp", [P, DB], dt.float32, kind="ExternalInput")
    y_d = nc.dram_tensor("yT", [D_OUT, C], dt.bfloat16, kind="ExternalOutput")

    with tile.TileContext(nc) as tc:
        with (
            tc.tile_pool(name="persist", bufs=1) as ppool,
            tc.tile_pool(name="yout", bufs=3) as ypool,
            tc.tile_pool(name="psum", bufs=8, space="PSUM") as pspool,
        ):
            x_tiles = [
                ppool.tile([P, C], dt.bfloat16, name=f"x{ki}", tag=f"x{ki}")
                for ki in range(KT)
            ]
            w01 = [
                ppool.tile([P, KW], dt.bfloat16, name=f"w{db}", tag=f"w{db}")
                for db in range(2)
            ]
            packs = [
                ppool.tile([P, 2 * KW], dt.bfloat16, name=f"wp{g}", tag=f"wp{g}")
                for g in range(1, 8)
            ]
            bt = ppool.tile([P, DB], dt.float32, name="bias", tag="bias")

            def lhsT(db, ki):
                if db < 2:
                    return w01[db][:, ki * P:(ki + 1) * P]
                g = db // 2
                off = (db - 2 * g) * KW + ki * P
                return packs[g - 1][:, off:off + P]

            # ── DMA schedule: two HWDGE rings, deadline-ordered ──────
            # ring A = sync, ring B = scalar. w0/w1 k0 slices first so
            # the first matmuls can start, then x tiles alternating
            # rings with the rest of w0/w1 threaded between, then the
            # 2-block weight packs.
            A, B = nc.sync, nc.scalar
            A.dma_start(w01[0][:, :P], wf_d[:, 0:P])
            B.dma_start(w01[1][:, :P], wf_d[:, KW:KW + P])
            A.dma_start(x_tiles[0][:], xT_d[0:P, :])
            B.dma_start(x_tiles[1][:], xT_d[P:2 * P, :])
            A.dma_start(w01[0][:, P:4 * P], wf_d[:, P:4 * P])
            B.dma_start(w01[1][:, P:4 * P], wf_d[:, KW + P:KW + 4 * P])
            A.dma_start(x_tiles[2][:], xT_d[2 * P:3 * P, :])
            B.dma_start(x_tiles[3][:], xT_d[3 * P:4 * P, :])
            A.dma_start(w01[0][:, 4 * P:], wf_d[:, 4 * P:KW])
            B.dma_start(w01[1][:, 4 * P:], wf_d[:, KW + 4 * P:2 * KW])
            A.dma_start(x_tiles[4][:], xT_d[4 * P:5 * P, :])
            B.dma_start(x_tiles[5][:], xT_d[5 * P:6 * P, :])
            A.dma_start(x_tiles[6][:], xT_d[6 * P:7 * P, :])
            B.dma_start(x_tiles[7][:], xT_d[7 * P:8 * P, :])
            B.dma_start(bt[:], bias_d[:])
            for g in range(1, 8):
                eng = A if g % 2 == 1 else B
                eng.dma_start(packs[g - 1][:], wf_d[:, 2 * g * KW:(2 * g + 2) * KW])

            # ── PE warmup: flip the HAM clock gate (~3.4us of activity)
            # while the first DMAs land. Warm tile memset on gpsimd so it
            # runs right after the framework's own preamble memsets.
            warm = ppool.tile([P, P], dt.bfloat16, name="warm", tag="warm")
            nc.gpsimd.memset(warm[:], 0.0)
            wps = pspool.tile([P, P], dt.float32, name="wps", tag="ps")
            for _ in range(34):
                nc.tensor.matmul(wps[:], warm[:], warm[:], start=True, stop=True)

            all_psums = {}

            def alloc_psums(db):
                all_psums[db] = [
                    pspool.tile([P, NFREE], dt.float32, name=f"ps{db}_{j}", tag="ps")
                    for j in range(len(chunk_of_db[db]))
                ]

            def emit_mms(db, ki):
                w = lhsT(db, ki)
                for j, (off, cw) in enumerate(chunk_of_db[db]):
                    nc.tensor.matmul(
                        all_psums[db][j][:, :cw],
                        w,
                        x_tiles[ki][:, off:off + cw],
                        start=(ki == 0),
                        stop=(ki == KT - 1),
                    )

            ep = 0

            def evict(db, chunk_order=None):
                nonlocal ep
                psums = all_psums[db]
                ysb = ypool.tile([P, C], dt.bfloat16, name="ysb", tag="ysb")
                bias_col = bt[:, db:db + 1]
                order = chunk_order or range(len(chunk_of_db[db]))
                for j in order:
                    off, cw = chunk_of_db[db][j]
                    if ep % 2 == 0:
                        nc.scalar.add(ysb[:, off:off + cw], psums[j][:, :cw], bias_col)
                    else:
                        nc.vector.tensor_scalar_add(
                            ysb[:, off:off + cw], psums[j][:, :cw], bias_col
                        )
                    ep += 1
                return ysb

            # db0/db1 interleaved by k-step through the x trickle.
            alloc_psums(0)
            alloc_psums(1)
            for ki in range(KT):
                emit_mms(0, ki)
                emit_mms(1, ki)

            for db in range(DB):
                if db >= 2:
                    alloc_psums(db)
                    for ki in range(KT):
                        emit_mms(db, ki)
                if db >= DB - 2:
                    # tail blocks: big chunks evicted first (one per
                    # engine), per-chunk DMAs on both rings so the last
                    # byte lands ASAP.
                    n = len(chunk_of_db[db])
                    order = list(range(n - 1)) + [n - 1] if n > 2 else None
                    ysb = evict(db, order)
                    for j, (off, cw) in enumerate(chunk_of_db[db]):
                        eng = nc.sync if (db + j) % 2 == 0 else nc.scalar
                        eng.dma_start(
                            y_d[db * P:(db + 1) * P, off:off + cw],
                            ysb[:, off:off + cw],
                        )
                else:
                    ysb = evict(db)
                    eng = nc.sync if db % 2 == 0 else nc.scalar
                    eng.dma_start(y_d[db * P:(db + 1) * P, :], ysb[:])

    nc.compile()
    return nc


def _run_spmd(in_maps, C, trace=False, trace_cores=None):
    from concourse.bass_utils import run_bass_kernel_spmd

    nc = _nc_cache.get(C)
    if nc is None:
        nc = _build_bass(C)
        _nc_cache[C] = nc
    return run_bass_kernel_spmd(
        nc,
        in_maps,
        core_ids=list(range(N_CORES)),
        trace=trace,
        trace_cores=trace_cores,
    )


def _prepare(x, weight, bias, ids_sorted):
    """Host-side routing: returns (in_maps, C, counts, starts)."""
    x = np.asarray(x)
    weight = np.asarray(weight)
    bias = np.asarray(bias)
    ids = np.asarray(ids_sorted)

    counts = np.bincount(ids, minlength=N_EXP).astype(np.int64)
    starts = np.zeros(N_EXP, dtype=np.int64)
    starts[1:] = np.cumsum(counts)[:-1]
    C = max(int(counts.max()), 2)
    C += C % 2

    KT = D_IN // P
    DB = D_OUT // P
    xb = x.astype(BF16)
    in_maps = []
    for e in range(N_EXP):
        n_e = int(counts[e])
        xeT = np.zeros((D_IN, C), dtype=BF16)
        if n_e:
            xeT[:, :n_e] = xb[starts[e]:starts[e] + n_e].T
        # flat weight: row p, col db*1024 + kt*128 + d  = W_e[db*128+d, kt*128+p]
        weT = weight[e].T.astype(BF16)  # [d_in, d_out]
        wf = np.ascontiguousarray(
            weT.reshape(KT, P, DB, P).transpose(1, 2, 0, 3)
        ).reshape(P, DB * KT * P)
        bp = np.ascontiguousarray(
            bias[e].astype(BF16).astype(np.float32).reshape(DB, P).T
        )
        in_maps.append({"xT": xeT, "wf": wf, "biasp": bp})
    return in_maps, C, counts, starts


def _assemble(results, counts, starts):
    out = np.empty((N_TOK, D_OUT), dtype=BF16)
    for e in range(N_EXP):
        n_e = int(counts[e])
        if n_e:
            out[starts[e]:starts[e] + n_e] = results[e]["yT"][:, :n_e].T
    return out


def kernel(x, weight, bias, ids_sorted):
    in_maps, C, counts, starts = _prepare(x, weight, bias, ids_sorted)
    res = _run_spmd(in_maps, C)
    return _assemble(res.results, counts, starts)


# revision 3
# speedup vs baseline: 1.0041x; 1.0041x over previous
"""MoE grouped linear (DMoELinear) on 8 Trainium2 NeuronCores.

Expert-parallel sharding: tokens are sorted by expert id, so expert e's
tokens form one contiguous slice. Core e receives expert e's tokens
(padded to a uniform capacity C = max group size, so all cores run one
SPMD NEFF), expert e's weight and bias, and computes
    yT_e = (x_e @ W_e.T).T.bf16 + b_e.bf16
with the weight block as the stationary matmul operand and tokens as
the moving free dim.

v2 schedule (trace-driven): the run is paced by DMA in the first third
and by the PE after, with a fixed ~8us NEFF epilogue. So: few, large
DMAs (each dma_start costs ~0.6us of issuing-engine time regardless of
size), deadline-ordered across the two HWDGE rings; weights flat
[128, DB*KT*128] so multi-block transfers are single 2D slices; w0/w1
split into k-slices only where the PE needs them early; remaining
weights as 2-block 512KB packs; x as 8 full-width k-tiles alternating
rings. db0/db1 interleave k-steps during the x trickle. Bias add fuses
into PSUM eviction (ACT/DVE alternating).
"""

import numpy as np
import ml_dtypes

N_TOK, D_IN, D_OUT, N_EXP = 8192, 1024, 2048, 8
N_CORES = 8
P = 128
NFREE = 512  # max matmul moving free dim (one PSUM bank of f32)

BF16 = ml_dtypes.bfloat16

_nc_cache: dict[int, object] = {}


def _chunks(C):
    out = []
    off = 0
    while off < C:
        cw = min(NFREE, C - off)
        out.append((off, cw))
        off += cw
    return out


def _build_bass(C: int):
    """Emit the per-core Bass/Tile kernel for token capacity C."""
    import concourse.bass as bass  # noqa: F401  (registers engines)
    import concourse.mybir as mybir
    import concourse.tile as tile
    from concourse import bacc

    dt = mybir.dt
    KT = D_IN // P      # 8 contraction tiles
    DB = D_OUT // P     # 16 output-row blocks
    KW = KT * P         # columns per db block in the flat weight (1024)
    chunks = _chunks(C)
    chunk_of_db = {db: chunks for db in range(DB)}

    nc = bacc.Bacc("TRN2", target_bir_lowering=False)

    # x k-tile ki lives in rows [ki*128, (ki+1)*128).
    xT_d = nc.dram_tensor("xT", [D_IN, C], dt.bfloat16, kind="ExternalInput")
    # flat weights: row p, col db*1024 + kt*128 + d  (lhsT slices are
    # contiguous 128-col blocks; multi-db packs are contiguous too).
    wf_d = nc.dram_tensor("wf", [P, DB * KW], dt.bfloat16, kind="ExternalInput")
    bias_d = nc.dram_tensor("biasp", [P, DB], dt.float32, kind="ExternalInput")
    y_d = nc.dram_tensor("yT", [D_OUT, C], dt.bfloat16, kind="ExternalOutput")

    with tile.TileContext(nc) as tc:
        with (
            tc.tile_pool(name="persist", bufs=1) as ppool,
            tc.tile_pool(name="yout", bufs=3) as ypool,
            tc.tile_pool(name="psum", bufs=8, space="PSUM") as pspool,
        ):
            x_tiles = [
                ppool.tile([P, C], dt.bfloat16, name=f"x{ki}", tag=f"x{ki}")
                for ki in range(KT)
            ]
            w_s = [
                ppool.tile([P, KW], dt.bfloat16, name=f"w{db}", tag=f"w{db}")
                for db in range(4)
            ]
            packs = [
                ppool.tile([P, 2 * KW], dt.bfloat16, name=f"wp{g}", tag=f"wp{g}")
                for g in range(2, 8)
            ]
            bt = ppool.tile([P, DB], dt.float32, name="bias", tag="bias")

            def lhsT(db, ki):
                if db < 4:
                    return w_s[db][:, ki * P:(ki + 1) * P]
                g = db // 2
                off = (db - 2 * g) * KW + ki * P
                return packs[g - 2][:, off:off + P]

            # ── DMA schedule: two HWDGE rings, deadline-ordered ──────
            # ring A = sync, ring B = scalar. w0/w1 k0 slices first so
            # the first matmuls can start, then x tiles alternating
            # rings with the rest of w0/w1 threaded between, then w2/w3
            # singles (db2 starts right after the trickle) and 2-block
            # packs for db4+ (those deadlines have huge slack).
            A, B = nc.sync, nc.scalar
            A.dma_start(w_s[0][:, :P], wf_d[:, 0:P])
            B.dma_start(w_s[1][:, :P], wf_d[:, KW:KW + P])
            A.dma_start(x_tiles[0][:], xT_d[0:P, :])
            B.dma_start(x_tiles[1][:], xT_d[P:2 * P, :])
            A.dma_start(w_s[0][:, P:4 * P], wf_d[:, P:4 * P])
            B.dma_start(w_s[1][:, P:4 * P], wf_d[:, KW + P:KW + 4 * P])
            A.dma_start(x_tiles[2][:], xT_d[2 * P:3 * P, :])
            B.dma_start(x_tiles[3][:], xT_d[3 * P:4 * P, :])
            A.dma_start(w_s[0][:, 4 * P:], wf_d[:, 4 * P:KW])
            B.dma_start(w_s[1][:, 4 * P:], wf_d[:, KW + 4 * P:2 * KW])
            A.dma_start(x_tiles[4][:], xT_d[4 * P:5 * P, :])
            B.dma_start(x_tiles[5][:], xT_d[5 * P:6 * P, :])
            A.dma_start(x_tiles[6][:], xT_d[6 * P:7 * P, :])
            B.dma_start(x_tiles[7][:], xT_d[7 * P:8 * P, :])
            B.dma_start(bt[:], bias_d[:])
            A.dma_start(w_s[2][:], wf_d[:, 2 * KW:3 * KW])
            A.dma_start(w_s[3][:], wf_d[:, 3 * KW:4 * KW])
            for g in range(2, 8):
                eng = B if g % 2 == 0 else A
                eng.dma_start(packs[g - 2][:], wf_d[:, 2 * g * KW:(2 * g + 2) * KW])

            # ── PE warmup: flip the HAM clock gate (~3.4us of activity)
            # while the first DMAs land.
            warm = ppool.tile([P, P], dt.bfloat16, name="warm", tag="warm")
            nc.vector.memset(warm[:], 0.0)
            wps = pspool.tile([P, P], dt.float32, name="wps", tag="ps")
            for _ in range(30):
                nc.tensor.matmul(wps[:], warm[:], warm[:], start=True, stop=True)

            all_psums = {}

            def alloc_psums(db):
                all_psums[db] = [
                    pspool.tile([P, NFREE], dt.float32, name=f"ps{db}_{j}", tag="ps")
                    for j in range(len(chunk_of_db[db]))
                ]

            def emit_mms(db, ki):
                w = lhsT(db, ki)
                for j, (off, cw) in enumerate(chunk_of_db[db]):
                    nc.tensor.matmul(
                        all_psums[db][j][:, :cw],
                        w,
                        x_tiles[ki][:, off:off + cw],
                        start=(ki == 0),
                        stop=(ki == KT - 1),
                    )

            ep = 0

            def evict(db, chunk_order=None):
                nonlocal ep
                psums = all_psums[db]
                ysb = ypool.tile([P, C], dt.bfloat16, name="ysb", tag="ysb")
                bias_col = bt[:, db:db + 1]
                order = chunk_order or range(len(chunk_of_db[db]))
                for j in order:
                    off, cw = chunk_of_db[db][j]
                    if ep % 2 == 0:
                        nc.scalar.add(ysb[:, off:off + cw], psums[j][:, :cw], bias_col)
                    else:
                        nc.vector.tensor_scalar_add(
                            ysb[:, off:off + cw], psums[j][:, :cw], bias_col
                        )
                    ep += 1
                return ysb

            # db0/db1 interleaved by k-step through the x trickle.
            alloc_psums(0)
            alloc_psums(1)
            for ki in range(KT):
                emit_mms(0, ki)
                emit_mms(1, ki)

            for db in range(DB):
                if db >= 2:
                    alloc_psums(db)
                    for ki in range(KT):
                        emit_mms(db, ki)
                if db >= DB - 2:
                    # tail blocks: big chunks evicted first (one per
                    # engine), per-chunk DMAs on both rings so the last
                    # byte lands ASAP.
                    n = len(chunk_of_db[db])
                    order = list(range(n - 1)) + [n - 1] if n > 2 else None
                    ysb = evict(db, order)
                    for j, (off, cw) in enumerate(chunk_of_db[db]):
                        eng = nc.sync if (db + j) % 2 == 0 else nc.scalar
                        eng.dma_start(
                            y_d[db * P:(db + 1) * P, off:off + cw],
                            ysb[:, off:off + cw],
                        )
                else:
                    ysb = evict(db)
                    eng = nc.sync if db % 2 == 0 else nc.scalar
                    eng.dma_start(y_d[db * P:(db + 1) * P, :], ysb[:])

    nc.compile()
    return nc


def _run_spmd(in_maps, C, trace=False, trace_cores=None):
    from concourse.bass_utils import run_bass_kernel_spmd

    nc = _nc_cache.get(C)
    if nc is None:
        nc = _build_bass(C)
        _nc_cache[C] = nc
    return run_bass_kernel_spmd(
        nc,
        in_maps,
        core_ids=list(range(N_CORES)),
        trace=trace,
        trace_cores=trace_cores,
    )


def _prepare(x, weight, bias, ids_sorted):
    """Host-side routing: returns (in_maps, C, counts, starts)."""
    x = np.asarray(x)
    weight = np.asarray(weight)
    bias = np.asarray(bias)
    ids = np.asarray(ids_sorted)

    counts = np.bincount(ids, minlength=N_EXP).astype(np.int64)
    starts = np.zeros(N_EXP, dtype=np.int64)
    starts[1:] = np.cumsum(counts)[:-1]
    C = max(int(counts.max()), 2)
    C += C % 2

    KT = D_IN // P
    DB = D_OUT // P
    xb = x.astype(BF16)
    in_maps = []
    for e in range(N_EXP):
        n_e = int(counts[e])
        xeT = np.zeros((D_IN, C), dtype=BF16)
        if n_e:
            xeT[:, :n_e] = xb[starts[e]:starts[e] + n_e].T
        # flat weight: row p, col db*1024 + kt*128 + d  = W_e[db*128+d, kt*128+p]
        weT = weight[e].T.astype(BF16)  # [d_in, d_out]
        wf = np.ascontiguousarray(
            weT.reshape(KT, P, DB, P).transpose(1, 2, 0, 3)
        ).reshape(P, DB * KT * P)
        bp = np.ascontiguousarray(
            bias[e].astype(BF16).astype(np.float32).reshape(DB, P).T
        )
        in_maps.append({"xT": xeT, "wf": wf, "biasp": bp})
    return in_maps, C, counts, starts


def _assemble(results, counts, starts):
    out = np.empty((N_TOK, D_OUT), dtype=BF16)
    for e in range(N_EXP):
        n_e = int(counts[e])
        if n_e:
            out[starts[e]:starts[e] + n_e] = results[e]["yT"][:, :n_e].T
    return out


def kernel(x, weight, bias, ids_sorted):
    in_maps, C, counts, starts = _prepare(x, weight, bias, ids_sorted)
    res = _run_spmd(in_maps, C)
    return _assemble(res.results, counts, starts)
